# revision 6
# baseline (speedup 1.0000x reference)
"""Trainium2 Bass kernel for the soft-logic cellular-automaton nn.Module.

Reference semantics (B=16, M=4096, N=8192, K=6, P=64, L=8, STEP=2):
    tw = sigmoid(toggle_gates)                      # (L, P, N)
    state = zeros(B, N); state[:, ::2] = x
    for l in range(L):
        win[b,n,i] = state[b, (n+i-2) mod N]        # i in 0..5
        w[b,n,p]   = prod_i (bit_i(p) ? win_i : 1-win_i)
        state[b,n] = clip(sum_p w[b,n,p]*tw[l,p,n], 0, 1)
    return state[:, ::2]

Sharding: grid dim N split across 8 cores (1024 owned columns each).
Each core computes a halo-grown region (2 left / 3 right per layer -> 16/24
total) so NO inter-core communication is needed during the 8 layers.

On-core layout ("F-major"): 128 partitions = (b=16) x (chunk c=8); each
partition holds a contiguous 168-column n-window (128 owned + 40 halo) on
the free dim, so the 6 convolution taps are plain AP column offsets.

Per layer the 64-term contraction  sum_p wA[p>>3]*wB[p&7]*tw[p,n]  runs on
the vector engine: 24 tree muls build wA[8]/wB[8], then per pa-group one
batched mul against tw (pb interleaved innermost) + one segmented
tensor_reduce, then a final mul+reduce over pa.  clip is skipped: tw in
(0.5, 0.732) and sum_p w = 1 exactly, so outputs stay inside (0,1).

toggle weights are streamed from DRAM per layer (b-replicated to the 128
partitions by 16 DMAs) and sigmoid-ed in place on the scalar engine,
double-buffered against compute.
"""

import os
import sys
from contextlib import ExitStack

import numpy as np

for _p in ("/opt/trn_rl_repo", "/root/.axon_site/_ro/trn_rl_repo"):
    if os.path.isdir(_p) and _p not in sys.path:
        sys.path.insert(0, _p)

import concourse.bass as bass  # noqa: E402
import concourse.tile as tile  # noqa: E402
from concourse import bacc, mybir  # noqa: E402
from concourse.bass_utils import run_bass_kernel_spmd  # noqa: E402

B, M, N, KK, P, L = 16, 4096, 8192, 6, 64, 8
NCORES = 8
NOWN = N // NCORES          # 1024 owned grid columns per core
NCHUNK = 8                  # chunks (partitions per batch row)
CHUNK = NOWN // NCHUNK      # 128 owned columns per partition
GROW_L, GROW_R = 2 * L, 3 * L   # 16, 24
W0 = CHUNK + GROW_L + GROW_R    # 168 column window at layer 0
XW = W0 // 2                    # 84 even columns carrying x
F32 = mybir.dt.float32


def _build_program(reps=1):
    nc = bacc.Bacc("TRN2", target_bir_lowering=False, debug=False)
    xs = nc.dram_tensor("xs", [128, XW], F32, kind="ExternalInput").ap()
    tg = nc.dram_tensor("tg", [L, NCHUNK, W0, P], F32, kind="ExternalInput").ap()
    out = nc.dram_tensor("out", [128, CHUNK // 2], F32, kind="ExternalOutput").ap()

    mult = mybir.AluOpType.mult
    add = mybir.AluOpType.add
    AX = mybir.AxisListType.X
    AF = mybir.ActivationFunctionType

    with tile.TileContext(nc) as tc, ExitStack() as ctx:
        pool = ctx.enter_context(tc.tile_pool(name="work", bufs=1))
        twpool = ctx.enter_context(tc.tile_pool(name="tw", bufs=2))

        S = [pool.tile([128, W0], F32, name="sA", tag="sA"),
             pool.tile([128, W0], F32, name="sB", tag="sB")]
        Cm = pool.tile([128, W0], F32, name="comp", tag="comp")
        tmp4 = pool.tile([128, W0, 4], F32, name="tmp4", tag="tmp4")
        wa = pool.tile([128, W0, 8], F32, name="wa", tag="wa")
        wb = pool.tile([128, W0, 8], F32, name="wb", tag="wb")
        prod = pool.tile([128, W0, 8], F32, name="prod", tag="prod")
        gf = pool.tile([128, W0, 8], F32, name="gf", tag="gf")

        # state init: zeros with x at even columns
        nc.vector.memset(S[0][:], 0.0)
        nc.sync.dma_start(out=S[0][:, 0:W0:2], in_=xs[:, :])

        tw_tiles = []

        def fetch_tw(layer):
            t = twpool.tile([128, W0, P], F32, name="twt", tag="tw")
            for b in range(16):
                nc.sync.dma_start(out=t[b * 8:(b + 1) * 8, :, :], in_=tg[layer % L])
            tw_tiles.append(t)

        def sigmoid_tw(layer):
            ll = layer % L
            lo, ro = 2 * ll + 2, W0 - 3 * ll - 3
            t = tw_tiles[layer]
            nc.scalar.activation(t[:, lo:ro, :], t[:, lo:ro, :], AF.Sigmoid)

        fetch_tw(0)
        sigmoid_tw(0)

        for gl in range(L * reps):
            l = gl % L
            lin, rin = 2 * l, W0 - 3 * l
            lo, ro = lin + 2, rin - 3
            wo = ro - lo
            sin, sout = S[gl % 2], S[(gl + 1) % 2]
            twl = tw_tiles[gl]

            # prefetch next layer's toggle gates (DMA early, sigmoid later)
            if gl + 1 < L * reps:
                fetch_tw(gl + 1)

            # comp = 1 - state on the input window (scalar engine)
            nc.scalar.activation(Cm[:, lin:rin], sin[:, lin:rin],
                                 AF.Identity, bias=1.0, scale=-1.0)

            # next layer's sigmoid queues on ACT behind comp
            if gl + 1 < L * reps:
                sigmoid_tw(gl + 1)

            def V(i, bit):
                buf = sin if bit else Cm
                return buf[:, lin + i: lin + i + wo]

            # wA tree over taps 0,1,2 (pa bit order: tap0 = MSB)
            for q in range(4):
                nc.vector.tensor_tensor(tmp4[:, 0:wo, q], V(0, q >> 1), V(1, q & 1), mult)
            for pa in range(8):
                nc.vector.tensor_tensor(wa[:, 0:wo, pa], tmp4[:, 0:wo, pa >> 1], V(2, pa & 1), mult)
            # wB tree over taps 3,4,5
            for q in range(4):
                nc.vector.tensor_tensor(tmp4[:, 0:wo, q], V(3, q >> 1), V(4, q & 1), mult)
            for pb in range(8):
                nc.vector.tensor_tensor(wb[:, 0:wo, pb], tmp4[:, 0:wo, pb >> 1], V(5, pb & 1), mult)

            # contraction: g[pa] = sum_pb wB[pb] * tw[pa*8+pb]
            for pa in range(8):
                nc.vector.tensor_tensor(prod[:, 0:wo, :], wb[:, 0:wo, :],
                                        twl[:, lo:ro, pa * 8:(pa + 1) * 8], mult)
                nc.vector.tensor_reduce(gf[:, 0:wo, pa], prod[:, 0:wo, 0:8],
                                        axis=AX, op=add)
            # out = sum_pa wA[pa] * g[pa]
            nc.vector.tensor_tensor(prod[:, 0:wo, :], wa[:, 0:wo, :], gf[:, 0:wo, :], mult)
            nc.vector.tensor_reduce(sout[:, lo:ro], prod[:, 0:wo, 0:8], axis=AX, op=add)

        # owned even columns -> output
        nc.sync.dma_start(out=out, in_=S[(L * reps) % 2][:, GROW_L:GROW_L + CHUNK:2])

    nc.compile()
    return nc


_prog_cache = {}


def _get_program(reps=1):
    key = ("nc", reps)
    if key not in _prog_cache:
        _prog_cache[key] = _build_program(reps)
    return _prog_cache[key]


def _shard_inputs(x, toggle_gates):
    x = np.ascontiguousarray(x, dtype=np.float32)
    tg = np.ascontiguousarray(toggle_gates, dtype=np.float32)
    in_maps = []
    c = np.arange(NCHUNK)
    j = np.arange(W0)
    for k in range(NCORES):
        n0 = k * NOWN
        nglob = (n0 + CHUNK * c[:, None] - GROW_L + j[None, :]) % N  # [8, 168]
        m_idx = nglob[:, 0::2] // 2                                   # [8, 84]
        xs = x[:, m_idx].reshape(B * NCHUNK, XW)                      # [128, 84]
        tgk = tg[:, :, nglob]                                         # [L, P, 8, 168]
        tgk = np.ascontiguousarray(tgk.transpose(0, 2, 3, 1))         # [L, 8, 168, P]
        in_maps.append({"xs": np.ascontiguousarray(xs), "tg": tgk})
    return in_maps


def _run(x, toggle_gates, trace=False, reps=1, **kw):
    nc = _get_program(reps)
    in_maps = _shard_inputs(x, toggle_gates)
    res = run_bass_kernel_spmd(nc, in_maps, list(range(NCORES)), trace=trace, **kw)
    y = np.empty((B, M), dtype=np.float32)
    for k in range(NCORES):
        o = np.asarray(res.results[k]["out"]).reshape(B, NCHUNK * CHUNK // 2)
        y[:, k * (NOWN // 2):(k + 1) * (NOWN // 2)] = o
    return y, res


def kernel(x, toggle_gates):
    y, _ = _run(x, toggle_gates)
    return y


# revision 7
# speedup vs baseline: 21.5583x; 21.5583x over previous
"""Trainium2 Bass kernel for the soft-logic cellular-automaton nn.Module.

Reference semantics (B=16, M=4096, N=8192, K=6, P=64, L=8, STEP=2):
    tw = sigmoid(toggle_gates)                      # (L, P, N)
    state = zeros(B, N); state[:, ::2] = x
    for l in range(L):
        win[b,n,i] = state[b, (n+i-2) mod N]        # i in 0..5
        w[b,n,p]   = prod_i (bit_i(p) ? win_i : 1-win_i)
        state[b,n] = clip(sum_p w[b,n,p]*tw[l,p,n], 0, 1)
    return state[:, ::2]

Sharding: grid dim N split across 8 cores (1024 owned columns each).
Each core computes a halo-grown region (2 left / 3 right per layer -> 16/24
total) so NO inter-core communication is needed during the 8 layers.

On-core layout ("F-major"): 128 partitions = (b=16) x (chunk c=8); each
partition holds a contiguous 168-column n-window (128 owned + 40 halo) on
the free dim, so the 6 convolution taps are plain AP column offsets.

Per layer the 64-term contraction  sum_p wA[p>>3]*wB[p&7]*tw[p,n]  runs on
the vector engine: 24 tree muls build wA[8]/wB[8], then per pa-group one
batched mul against tw (pb interleaved innermost) + one segmented
tensor_reduce, then a final mul+reduce over pa.  clip is skipped: tw in
(0.5, 0.732) and sum_p w = 1 exactly, so outputs stay inside (0,1).

toggle weights are streamed from DRAM per layer (b-replicated to the 128
partitions by 16 DMAs) and sigmoid-ed in place on the scalar engine,
double-buffered against compute.
"""

import os
import sys
from contextlib import ExitStack

import numpy as np

for _p in ("/opt/trn_rl_repo", "/root/.axon_site/_ro/trn_rl_repo"):
    if os.path.isdir(_p) and _p not in sys.path:
        sys.path.insert(0, _p)

import concourse.bass as bass  # noqa: E402
import concourse.tile as tile  # noqa: E402
from concourse import bacc, mybir  # noqa: E402
from concourse.bass_utils import run_bass_kernel_spmd  # noqa: E402

B, M, N, KK, P, L = 16, 4096, 8192, 6, 64, 8
NCORES = 8
NOWN = N // NCORES          # 1024 owned grid columns per core
NCHUNK = 8                  # chunks (partitions per batch row)
CHUNK = NOWN // NCHUNK      # 128 owned columns per partition
GROW_L, GROW_R = 2 * L, 3 * L   # 16, 24
W0 = CHUNK + GROW_L + GROW_R    # 168 column window at layer 0
XW = W0 // 2                    # 84 even columns carrying x
F32 = mybir.dt.float32


def _build_program(reps=1):
    nc = bacc.Bacc("TRN2", target_bir_lowering=False, debug=False)
    xs = nc.dram_tensor("xs", [128, XW], F32, kind="ExternalInput").ap()
    tg = nc.dram_tensor("tg", [L, NCHUNK, W0, P], F32, kind="ExternalInput").ap()
    out = nc.dram_tensor("out", [128, CHUNK // 2], F32, kind="ExternalOutput").ap()

    mult = mybir.AluOpType.mult
    add = mybir.AluOpType.add
    AX = mybir.AxisListType.X
    AF = mybir.ActivationFunctionType

    with tile.TileContext(nc) as tc, ExitStack() as ctx:
        pool = ctx.enter_context(tc.tile_pool(name="work", bufs=1))
        twpool = ctx.enter_context(tc.tile_pool(name="tw", bufs=2))

        S = [pool.tile([128, W0], F32, name="sA", tag="sA"),
             pool.tile([128, W0], F32, name="sB", tag="sB")]
        Cm = pool.tile([128, W0], F32, name="comp", tag="comp")
        tmp4 = pool.tile([128, W0, 4], F32, name="tmp4", tag="tmp4")
        wa = pool.tile([128, W0, 8], F32, name="wa", tag="wa")
        wb = pool.tile([128, W0, 8], F32, name="wb", tag="wb")
        prod = pool.tile([128, W0, 8], F32, name="prod", tag="prod")
        gf = pool.tile([128, W0, 8], F32, name="gf", tag="gf")

        # state init: zeros with x at even columns
        nc.vector.memset(S[0][:], 0.0)
        nc.sync.dma_start(out=S[0][:, 0:W0:2], in_=xs[:, :])

        tw_tiles = []

        def fetch_tw(layer):
            t = twpool.tile([128, W0, P], F32, name="twt", tag="tw")
            # single DMA, b-replication via 0-stride src dim -> all 16 SDMA
            # engines engaged (the 16 narrow per-b DMAs serialized badly)
            nc.sync.dma_start(out=t[:, :, :], in_=tg[layer % L].partition_broadcast(16))
            tw_tiles.append(t)

        def sigmoid_tw(layer):
            ll = layer % L
            lo, ro = 2 * ll + 2, W0 - 3 * ll - 3
            t = tw_tiles[layer]
            nc.scalar.activation(t[:, lo:ro, :], t[:, lo:ro, :], AF.Sigmoid)

        fetch_tw(0)
        sigmoid_tw(0)

        for gl in range(L * reps):
            l = gl % L
            lin, rin = 2 * l, W0 - 3 * l
            lo, ro = lin + 2, rin - 3
            wo = ro - lo
            sin, sout = S[gl % 2], S[(gl + 1) % 2]
            twl = tw_tiles[gl]

            # prefetch next layer's toggle gates (DMA early, sigmoid later)
            if gl + 1 < L * reps:
                fetch_tw(gl + 1)

            # comp = 1 - state on the input window (scalar engine)
            nc.scalar.activation(Cm[:, lin:rin], sin[:, lin:rin],
                                 AF.Identity, bias=1.0, scale=-1.0)

            # next layer's sigmoid queues on ACT behind comp
            if gl + 1 < L * reps:
                sigmoid_tw(gl + 1)

            def V(i, bit):
                buf = sin if bit else Cm
                return buf[:, lin + i: lin + i + wo]

            # wA tree over taps 0,1,2 (pa bit order: tap0 = MSB)
            for q in range(4):
                nc.vector.tensor_tensor(tmp4[:, 0:wo, q], V(0, q >> 1), V(1, q & 1), mult)
            for pa in range(8):
                nc.vector.tensor_tensor(wa[:, 0:wo, pa], tmp4[:, 0:wo, pa >> 1], V(2, pa & 1), mult)
            # wB tree over taps 3,4,5
            for q in range(4):
                nc.vector.tensor_tensor(tmp4[:, 0:wo, q], V(3, q >> 1), V(4, q & 1), mult)
            for pb in range(8):
                nc.vector.tensor_tensor(wb[:, 0:wo, pb], tmp4[:, 0:wo, pb >> 1], V(5, pb & 1), mult)

            # contraction: g[pa] = sum_pb wB[pb] * tw[pa*8+pb]
            for pa in range(8):
                nc.vector.tensor_tensor(prod[:, 0:wo, :], wb[:, 0:wo, :],
                                        twl[:, lo:ro, pa * 8:(pa + 1) * 8], mult)
                nc.vector.tensor_reduce(gf[:, 0:wo, pa], prod[:, 0:wo, 0:8],
                                        axis=AX, op=add)
            # out = sum_pa wA[pa] * g[pa]
            nc.vector.tensor_tensor(prod[:, 0:wo, :], wa[:, 0:wo, :], gf[:, 0:wo, :], mult)
            nc.vector.tensor_reduce(sout[:, lo:ro], prod[:, 0:wo, 0:8], axis=AX, op=add)

        # owned even columns -> output
        nc.sync.dma_start(out=out, in_=S[(L * reps) % 2][:, GROW_L:GROW_L + CHUNK:2])

    nc.compile()
    return nc


_prog_cache = {}


def _get_program(reps=1):
    key = ("nc", reps)
    if key not in _prog_cache:
        _prog_cache[key] = _build_program(reps)
    return _prog_cache[key]


def _shard_inputs(x, toggle_gates):
    x = np.ascontiguousarray(x, dtype=np.float32)
    tg = np.ascontiguousarray(toggle_gates, dtype=np.float32)
    in_maps = []
    c = np.arange(NCHUNK)
    j = np.arange(W0)
    for k in range(NCORES):
        n0 = k * NOWN
        nglob = (n0 + CHUNK * c[:, None] - GROW_L + j[None, :]) % N  # [8, 168]
        m_idx = nglob[:, 0::2] // 2                                   # [8, 84]
        xs = x[:, m_idx].reshape(B * NCHUNK, XW)                      # [128, 84]
        tgk = tg[:, :, nglob]                                         # [L, P, 8, 168]
        tgk = np.ascontiguousarray(tgk.transpose(0, 2, 3, 1))         # [L, 8, 168, P]
        in_maps.append({"xs": np.ascontiguousarray(xs), "tg": tgk})
    return in_maps


def _run(x, toggle_gates, trace=False, reps=1, **kw):
    nc = _get_program(reps)
    in_maps = _shard_inputs(x, toggle_gates)
    res = run_bass_kernel_spmd(nc, in_maps, list(range(NCORES)), trace=trace, **kw)
    y = np.empty((B, M), dtype=np.float32)
    for k in range(NCORES):
        o = np.asarray(res.results[k]["out"]).reshape(B, NCHUNK * CHUNK // 2)
        y[:, k * (NOWN // 2):(k + 1) * (NOWN // 2)] = o
    return y, res


def kernel(x, toggle_gates):
    y, _ = _run(x, toggle_gates)
    return y
